# revision 8
# baseline (speedup 1.0000x reference)
"""Expert-parallel grouped GEMM (MoE) kernel for Trainium2.

Problem: inputs [65536, 1024] sorted by expert (8192 tokens/expert),
weight [8, 512, 1024]; out[t] = x[t] @ W[expert(t)].T -> [65536, 512].

Sharding: expert-parallel across 8 NeuronCores. Tokens are already sorted
by expert and expert_size is static, so core e simply takes token rows
[e*8192:(e+1)*8192] and weight[e] - no all-to-all needed.

Device kernel (per core): one [8192,1024] @ [1024,512] GEMM.

v3: all-fp8 DoubleRow with residual-correction matmuls ("fp8c").
DoubleRow packs 2 e4m3 weights per PE cell: a [256k x 128m] x [256k,
512n] matmul streams 512 output columns at 0.5 cycles/row - 4x the
fp16 rate per unit contraction. Pure e4m3 is 3.8e-2 rel err, so the
error is repaired with *more fp8 matmuls* instead of fp16:

    out = xh@wh + xlo@wh (first GX_KP k-pairs) + xh@wlo (GW_KP pairs)

where xh=e4m3(x), xlo=e4m3(x-xh), wh=e4m3(32w), wlo=e4m3(32w-wh).
The residuals are ~3% the magnitude of the data, landing in e4m3's
subnormal/small-normal range where their quantization error is ~2% of
the residual, i.e. ~6e-4 of the data. All three terms carry the same
2^5 scale, so they accumulate into ONE PSUM bank and the single
PSUM->SBUF copy applies 1/32 (tensor_scalar_mul). Measured-sim error:
gx=2: 1.88e-2 (10 MM/tile), gx=3: 1.34e-2 (11), gx=4: 1.3e-3 (12),
vs the 2e-2 gate; PE model 68/75/82us vs 112us fp16 baseline.

Structure (wstat): stationary = w k-pair tile [128,2,128] (DoubleRow
3D AP), moving = x [128,2,CHUNK]; psum [o-tile 128, CHUNK] fp32 so the
output leaves transposed (outT [O,S], host transposes back). Stationary
reuse: per (o,kp) the main c-loop and x-corr c-loop share wh[kp,o] and
the w-corr c-loop uses wlo[kp,o] - bacc emits one LDWEIGHTS per matmul
(DoubleRow LDW = 256 cols = 213ns > 107ns matmul, which would be
LDWEIGHTS-bound), so a post-compile pass strips LDWEIGHTS whose weight
AP matches the previous one on the PE stream (8 LDW per 40 MM after
dedup, fully hidden). w tiles stay resident in SBUF; x streams in
double-buffered chunk-groups on the SP HWDGE ring; outputs leave on
the ACT ring.
"""

import numpy as np

E = 8          # experts == cores
O = 512        # out_features
I = 1024       # in_features
S = 8192       # tokens per expert
KT = 8         # 128-row k-tiles
KP = 4         # DoubleRow k-pairs (256 contraction each)
GX_KP = 2      # k-pairs with x-residual correction (err 1.88e-2 @ 2)
GW_KP = 4      # k-pairs with w-residual correction
WSCALE = 32.0  # power-of-2 weight pre-scale (undone in the PSUM copy)
CHUNK = 512    # tokens per matmul (psum bank = 512 fp32)
CG = 4         # chunks per group (psum half, so groups double-buffer)
CG_TOK = CG * CHUNK
N_CG = S // CG_TOK
OT = O // 128  # o-tiles
X_BUFS = 2     # chunk-group buffers (prefetch depth)
DEDUP = True   # strip redundant LDWEIGHTS post-compile (required: wstat
               # repeats the stationary 4-8x and DR LDW is 2x MM time)

_cache = {}


def _dedup_ldweights(nc):
    """Remove InstLdweights that reload the identical weight tile.

    Tracks the last-loaded weight signature along each block's PE stream;
    resets at any PE instruction other than a plain matmul (branches,
    drains, barriers, transposes), so loop back-edges stay conservative.
    Waits/updates of removed loads move to the next kept PE instruction.

    Semaphore waits are `sem-ge-imm` against monotonic counters, so a
    per-block watermark of values this PE stream has already waited on
    lets redundant carried waits be dropped instead of accumulating past
    the per-instruction wait-slot limit (walrus "Too many sync wait
    commands"). Non-ge waits (barrier eq-waits) are never dropped.
    """
    from concourse import mybir

    removed = 0
    for fn in nc.m.functions:
        for blk in fn.blocks:
            insts = blk.instructions
            keep = []
            last_sig = None
            mark = {}  # sem id -> max value already waited on (PE stream)

            def covered(si):
                if si is None:
                    return True
                if si.on_update:
                    return False
                for wt in si.on_wait:
                    if (getattr(wt, "sync_type", None) != "semaphore"
                            or getattr(wt, "wait_mode", None) != "sem-ge-imm"
                            or mark.get(wt.id, -1) < wt.wait_value):
                        return False
                return True

            for inst in insts:
                if inst.engine != mybir.EngineType.PE:
                    keep.append(inst)
                    continue
                if isinstance(inst, mybir.InstLdweights) and not inst.is_transpose:
                    a = inst.ins[0]
                    sig = (a.memref, a.offset, str(a.ap),
                           str(inst.tile_position), str(inst.perf_mode))
                    if sig == last_sig and covered(inst.sync_info):
                        removed += 1
                        continue
                    last_sig = sig
                elif not (isinstance(inst, mybir.InstMatmult)
                          and not inst.is_transpose):
                    last_sig = None
                si = inst.sync_info
                if si is not None:
                    for wt in si.on_wait:
                        if (getattr(wt, "sync_type", None) == "semaphore"
                                and getattr(wt, "wait_mode", None) == "sem-ge-imm"):
                            if mark.get(wt.id, -1) < wt.wait_value:
                                mark[wt.id] = wt.wait_value
                keep.append(inst)
            insts[:] = keep
    return removed


def _build_nc(repeats=1, loop=0, idle=0):
    import concourse.bass as bass
    import concourse.tile as tile
    from concourse import bacc, mybir
    from contextlib import nullcontext

    f8 = mybir.dt.float8e4
    DR = mybir.MatmulPerfMode.DoubleRow

    nc = bacc.Bacc("TRN2", target_bir_lowering=False, debug=False)
    xhT = nc.dram_tensor("xhT", [I, S], f8, kind="ExternalInput")
    xloT = nc.dram_tensor("xloT", [GX_KP * 256, S], f8, kind="ExternalInput")
    whT = nc.dram_tensor("whT", [I, O], f8, kind="ExternalInput")
    wloT = nc.dram_tensor("wloT", [GW_KP * 256, O], f8, kind="ExternalInput")
    outT = nc.dram_tensor("out", [O, S], mybir.dt.float16, kind="ExternalOutput")
    if idle:
        ping = nc.dram_tensor("ping", [1, 8], mybir.dt.float16)
        pong = nc.dram_tensor("pong", [1, 8], mybir.dt.float16)

    with tile.TileContext(nc) as tc:
        with (
            tc.tile_pool(name="wpool", bufs=1) as wpool,
            tc.tile_pool(name="xpool", bufs=X_BUFS) as xpool,
            tc.tile_pool(name="opool", bufs=4) as opool,
            tc.tile_pool(name="psum", bufs=8, space=bass.MemorySpace.PSUM) as psum_pool,
        ):
            wt = wpool.tile([128, KT, O], f8, name="wt")
            wlo_t = wpool.tile([128, GW_KP * 2, O], f8, name="wlo_t")

            def load_cg(cgi, with_weights=False):
                # k-pair-ordered interleave so (o=0, kp) compute can chase
                # the loads: each kp needs only its 2 xh stripes (+2 xlo).
                t0 = cgi * CG_TOK
                xh = xpool.tile([128, KT, CG_TOK], f8, tag="xh", name="xh")
                xlo = xpool.tile([128, GX_KP * 2, CG_TOK], f8, tag="xlo",
                                 name="xlo")
                for kp in range(KP):
                    for j in (2 * kp, 2 * kp + 1):
                        if with_weights:
                            nc.sync.dma_start(wt[:, j, :],
                                              whT[j * 128:(j + 1) * 128, :])
                            if j < GW_KP * 2:
                                nc.sync.dma_start(
                                    wlo_t[:, j, :],
                                    wloT[j * 128:(j + 1) * 128, :])
                        nc.sync.dma_start(xh[:, j, :],
                                          xhT[j * 128:(j + 1) * 128,
                                              t0:t0 + CG_TOK])
                        if j < GX_KP * 2:
                            nc.sync.dma_start(
                                xlo[:, j, :],
                                xloT[j * 128:(j + 1) * 128, t0:t0 + CG_TOK])
                return xh, xlo

            last_ot = [None]

            def compute_cg(cgi, xh, xlo):
                t0 = cgi * CG_TOK
                for o in range(OT):
                    pss = [psum_pool.tile([128, CHUNK], mybir.dt.float32,
                                          name="ps", tag="ps")
                           for _ in range(CG)]
                    for kp in range(KP):
                        wm = wt[:, 2 * kp:2 * kp + 2, o * 128:(o + 1) * 128]
                        for c in range(CG):
                            nc.tensor.matmul(
                                pss[c][:], wm,
                                xh[:, 2 * kp:2 * kp + 2,
                                   c * CHUNK:(c + 1) * CHUNK],
                                start=(kp == 0), stop=False, perf_mode=DR)
                        if kp < GX_KP:
                            for c in range(CG):
                                nc.tensor.matmul(
                                    pss[c][:], wm,
                                    xlo[:, 2 * kp:2 * kp + 2,
                                        c * CHUNK:(c + 1) * CHUNK],
                                    start=False, stop=False, perf_mode=DR)
                        if kp < GW_KP:
                            wc = wlo_t[:, 2 * kp:2 * kp + 2,
                                       o * 128:(o + 1) * 128]
                            for c in range(CG):
                                nc.tensor.matmul(
                                    pss[c][:], wc,
                                    xh[:, 2 * kp:2 * kp + 2,
                                       c * CHUNK:(c + 1) * CHUNK],
                                    start=False, stop=(kp == KP - 1),
                                    perf_mode=DR)
                    ot = opool.tile([128, CG, CHUNK], mybir.dt.float16,
                                    tag="ot", name="ot")
                    for c in range(CG):
                        nc.vector.tensor_scalar_mul(ot[:, c, :], pss[c][:],
                                                    1.0 / WSCALE)
                    nc.scalar.dma_start(
                        outT[o * 128:(o + 1) * 128, t0:t0 + CG_TOK], ot[:])
                    last_ot[0] = ot[:, 0, :]

            loop_cm = (
                tc.For_i(0, loop, 1,
                         hint_engines=(mybir.EngineType.PE, mybir.EngineType.SP,
                                       mybir.EngineType.DVE))
                if loop else nullcontext()
            )
            with loop_cm:
                for _ in range(repeats):
                    pending = []
                    for cgi in range(N_CG):
                        pending.append((cgi, *load_cg(cgi, with_weights=cgi == 0)))
                        if len(pending) >= X_BUFS:
                            compute_cg(*pending.pop(0))
                    for args in pending:
                        compute_cg(*args)
                # low-power idle: dependent tiny DMA ping-pong through one
                # SBUF tile; first copy reads the gemm's final output tile
                # so the idle runs strictly AFTER the gemm. Keeps average
                # chip power low so duty-cycled benchmarks see the
                # unthrottled PE clock.
                if idle:
                    idle_t = wpool.tile([1, 8], mybir.dt.float16, name="idle_t")
                    if last_ot[0] is not None:
                        nc.sync.dma_start(idle_t[:], last_ot[0][0:1, 0:8])
                    for i in range(idle):
                        if i % 2 == 0:
                            nc.sync.dma_start(pong[:], idle_t[:])
                        else:
                            nc.sync.dma_start(idle_t[:], ping[:])
    nc.compile()
    if DEDUP and repeats > 0:
        _dedup_ldweights(nc)
    return nc


def _get_nc(repeats=1, loop=0, idle=0):
    key = (repeats, loop, idle, GX_KP, GW_KP, CHUNK, CG, X_BUFS, DEDUP)
    if key not in _cache:
        _cache[key] = _build_nc(repeats, loop, idle)
    return _cache[key]


def make_in_maps(inputs, weight):
    """Per-core input tensors: quantized hi/lo splits, transposed."""
    import ml_dtypes

    f8 = ml_dtypes.float8_e4m3
    in_maps = []
    for e in range(E):
        x_e = inputs[e * S:(e + 1) * S, :]          # [S, I] fp32
        w_e = weight[e] * WSCALE                    # [O, I] fp32, pre-scaled
        xh = x_e.astype(f8)
        xlo = (x_e - xh.astype(np.float32))[:, :GX_KP * 256].astype(f8)
        wh = w_e.astype(f8)
        wlo = (w_e - wh.astype(np.float32))[:, :GW_KP * 256].astype(f8)
        in_maps.append({
            "xhT": np.ascontiguousarray(xh.T),
            "xloT": np.ascontiguousarray(xlo.T),
            "whT": np.ascontiguousarray(wh.T),
            "wloT": np.ascontiguousarray(wlo.T),
        })
    return in_maps


def run(inputs, weight, trace=False, repeats=1, loop=0):
    """Shard, run on 8 cores, gather. Returns (out, BassKernelResults)."""
    from concourse.bass_utils import run_bass_kernel_spmd

    nc = _get_nc(repeats, loop)
    in_maps = make_in_maps(inputs, weight)
    res = run_bass_kernel_spmd(nc, in_maps, list(range(E)), trace=trace)
    outs = [res.results[e]["out"].T for e in range(E)]   # [S, O] each
    out = np.concatenate([o.astype(np.float32) for o in outs], axis=0)
    return out, res


def kernel(inputs, weight, expert_size):
    inputs = np.asarray(inputs, dtype=np.float32)
    weight = np.asarray(weight, dtype=np.float32)
    assert inputs.shape == (E * S, I) and weight.shape == (E, O, I)
    assert int(expert_size) == S
    out, _ = run(inputs, weight, trace=False)
    return out


# revision 14
# speedup vs baseline: 2.2613x; 2.2613x over previous
"""Expert-parallel grouped GEMM (MoE) kernel for Trainium2.

Problem: inputs [65536, 1024] sorted by expert (8192 tokens/expert),
weight [8, 512, 1024]; out[t] = x[t] @ W[expert(t)].T -> [65536, 512].

Sharding: expert-parallel across 8 NeuronCores. Tokens are already sorted
by expert and expert_size is static, so core e simply takes token rows
[e*8192:(e+1)*8192] and weight[e] - no all-to-all needed.

Device kernel (per core): one [8192,1024] @ [1024,512] GEMM.

v2: hybrid fp16 + fp8-DoubleRow split-K. The PE streams one moving
column per cycle for 2-byte dtypes, so the fp16 kernel is pinned at
~109us (262144 col-cycles @ 2.4GHz). DoubleRow packs 2 fp8 weights per
PE cell (contraction 256 per matmul at ~1.13 cyc/row measured, i.e.
1.77x fp16 per contraction tile), but pure e4m3 costs 3.8e-2 rel err
(> 2e-2 gate). Hybrid: contraction k=0..767 in fp16 (6 k-tiles),
k=768..1023 as ONE DoubleRow fp8 matmul (2 packed k-tiles), both
accumulating into the same PSUM bank. Error 1.88e-2 on HW (numpy
e4m3-RNE sim matches to 0.1%); measured ~91us vs 112us baseline.
f=0.25 is the error-budget optimum of the whole fp8-mix family at the
measured DR rate (residual-correction variants cost 0.565x fp16 per
corrected tile and lose; an all-fp8 + corrections kernel measured
208us). A 12-matmul scratch warm-up at each iteration start releases
the PE-HAM 4/8 clock gate during the first x-block's DMA (each
measurement iteration starts cold behind a >3.4us idle); it won every
paired A/B cycle.

- Scale trick: w rows ~ N(0, 1/1024) land in e4m3 subnormals, so both
  w16 and w8 are pre-scaled by 32 host-side (exact power of two; w*32 ~
  N(0,1) is centered for e4m3). PSUM then holds 32*out and the final
  PSUM->SBUF copy becomes tensor_scalar_mul(1/32) - same DVE cost as
  the tensor_copy it replaces.
- Both DoubleRow operands are 3D APs [128, 2, F]: contraction index =
  j*128 + p for subtile j, partition p - i.e. two adjacent k-tiles in
  the same k-major SBUF layout the fp16 tiles already use.
- x stationary per token-tile ([128,2,128] for fp8, LDWEIGHTS 256 cols,
  no FWL); w moving ([128,2,512] fp8 / [128,512] fp16). Weight tiles
  stay resident in SBUF; x streams in prefetched blocks.
"""

import numpy as np

E = 8          # experts == cores
O = 512        # out_features
I = 1024       # in_features
S = 8192       # tokens per expert
K16T = 6       # fp16 k-tiles (contraction 0..767)
K16 = K16T * 128
K8T = 2        # fp8 k-tiles packed into one DoubleRow matmul (768..1023)
K8 = K8T * 128
WSCALE = 32.0  # power-of-2 weight pre-scale (undone in the PSUM copy)
S_BLK = 2048   # max tokens per streamed x block
BLOCKS = (512, 1536, 2048, 2048, 1536, 512)  # ramp up AND down, sums to S
X_BUFS = 4     # x block buffers (prefetch depth)
FP8 = True     # hybrid split-K; False reproduces the all-fp16 baseline
OUT_B = 4      # t-tiles batched per output DMA
WARMUP = 12    # HAM warm-up matmuls at iteration start (0 disables)

assert K16 + (K8 if FP8 else 0) == I or not FP8
if not FP8:
    K16T, K16 = 8, 1024  # plain fp16 over the full contraction

_cache = {}


def _build_nc(repeats=1, loop=0, idle=0):
    import concourse.bass as bass
    import concourse.tile as tile
    from concourse import bacc, mybir
    from contextlib import nullcontext

    in_dt = mybir.dt.float16
    f8_dt = mybir.dt.float8e4
    blocks = []  # (start_token, n_tokens)
    pos = 0
    for sz in BLOCKS:
        blocks.append((pos, sz))
        pos += sz
    assert pos == S and all(sz % 128 == 0 and sz <= S_BLK for _, sz in blocks)

    nc = bacc.Bacc("TRN2", target_bir_lowering=False, debug=False)
    xT = nc.dram_tensor("xT", [K16, S], in_dt, kind="ExternalInput")
    wT = nc.dram_tensor("wT", [K16, O], in_dt, kind="ExternalInput")
    if FP8:
        x8T = nc.dram_tensor("x8T", [K8, S], f8_dt, kind="ExternalInput")
        w8T = nc.dram_tensor("w8T", [K8, O], f8_dt, kind="ExternalInput")
    outT = nc.dram_tensor("out", [S, O], mybir.dt.float16, kind="ExternalOutput")
    if idle:
        ping = nc.dram_tensor("ping", [1, 8], mybir.dt.float16)
        pong = nc.dram_tensor("pong", [1, 8], mybir.dt.float16)

    with tile.TileContext(nc) as tc:
        with (
            tc.tile_pool(name="wpool", bufs=1) as wpool,
            tc.tile_pool(name="xpool", bufs=X_BUFS) as xpool,
            tc.tile_pool(name="opool", bufs=4) as opool,
            tc.tile_pool(name="psum", bufs=8, space=bass.MemorySpace.PSUM) as psum_pool,
        ):
            wt = wpool.tile([128, K16T * O], in_dt)
            w8t = (wpool.tile([128, K8T, O], f8_dt, name="w8t")
                   if FP8 else None)
            if WARMUP:
                # Scratch operand for HAM warm-up matmuls (content irrelevant;
                # the scratch psum result is never read).
                wrm = wpool.tile([128, O], in_dt, name="wrm")
                nc.vector.memset(wrm[:], 0)

            def load_block(blk, with_weights=False):
                # with_weights: interleave the resident-weight k-tile loads
                # with this block's stripes so the first matmul (needs only
                # wt[k=0] + stripe[k=0]) starts ~5us earlier than with a
                # serial full-weight prefix.
                s0, sz = blk
                xblk = xpool.tile([128, K16T * sz], in_dt, tag="xblk")
                x8blk = (xpool.tile([128, K8T, sz], f8_dt, tag="x8blk",
                                    name="x8blk")
                         if FP8 else None)
                for k in range(K16T):
                    if with_weights:
                        nc.sync.dma_start(wt[:, k * O:(k + 1) * O],
                                          wT[k * 128:(k + 1) * 128, :])
                    nc.sync.dma_start(
                        xblk[:, k * sz:(k + 1) * sz],
                        xT[k * 128:(k + 1) * 128, s0:s0 + sz],
                    )
                if FP8:
                    for j in range(K8T):
                        if with_weights:
                            nc.sync.dma_start(w8t[:, j, :],
                                              w8T[j * 128:(j + 1) * 128, :])
                        nc.sync.dma_start(
                            x8blk[:, j, :],
                            x8T[j * 128:(j + 1) * 128, s0:s0 + sz],
                        )
                return xblk, x8blk

            last_ot = [None]

            def compute_block(blk, xblk, x8blk):
                s0, sz = blk
                for tg in range(sz // 128 // OUT_B):
                    ot = opool.tile([128, OUT_B, O], mybir.dt.float16, tag="ot")
                    for ti in range(OUT_B):
                        t = tg * OUT_B + ti
                        ps = psum_pool.tile([128, O], mybir.dt.float32,
                                            name="ps", tag="ps")
                        for k in range(K16T):
                            nc.tensor.matmul(
                                ps[:],
                                xblk[:, k * sz + t * 128: k * sz + (t + 1) * 128],
                                wt[:, k * O:(k + 1) * O],
                                start=(k == 0),
                                stop=(k == K16T - 1) and not FP8,
                            )
                        if FP8:
                            nc.tensor.matmul(
                                ps[:],
                                x8blk[:, :, t * 128:(t + 1) * 128],
                                w8t[:],
                                start=False,
                                stop=True,
                                perf_mode=mybir.MatmulPerfMode.DoubleRow,
                            )
                        nc.vector.tensor_scalar_mul(ot[:, ti, :], ps[:],
                                                    1.0 / WSCALE)
                    g0 = s0 + tg * OUT_B * 128
                    dst = outT[g0:g0 + OUT_B * 128, :].rearrange(
                        "(t p) o -> p t o", p=128)
                    nc.scalar.dma_start(dst, ot[:])
                    last_ot[0] = ot[:, 0, :]

            loop_cm = (
                tc.For_i(0, loop, 1,
                         hint_engines=(mybir.EngineType.PE, mybir.EngineType.SP,
                                       mybir.EngineType.DVE))
                if loop else nullcontext()
            )
            with loop_cm:
                for _ in range(repeats):
                    if WARMUP:
                        # PE-HAM releases its 4/8 clock gate only after ~3.4us
                        # of sustained PE activity; each For_i iteration starts
                        # cold (the idle chain is a >3.4us PE gap). These
                        # no-dependency matmuls run during the first x-block's
                        # DMA (otherwise PE-idle time), so the real stream
                        # starts at 2.4GHz instead of ramping mid-stream.
                        pwarm = psum_pool.tile([128, O], mybir.dt.float32,
                                               name="pwarm", tag="ps")
                        for i in range(WARMUP):
                            nc.tensor.matmul(
                                pwarm[:], wrm[:, 0:128], wrm[:],
                                start=(i == 0), stop=(i == WARMUP - 1))
                    pending = []  # (blk, xblk, x8blk) loaded but not computed
                    for bi, blk in enumerate(blocks):
                        pending.append((blk, *load_block(blk, with_weights=bi == 0)))
                        if len(pending) >= X_BUFS:
                            compute_block(*pending.pop(0))
                    for args in pending:
                        compute_block(*args)
                # low-power idle: dependent tiny DMA ping-pong through one
                # SBUF tile (Tile tracks the tile's RAW/WAR deps, so the
                # copies serialize on each other's completion latency).
                # The first copy reads the gemm's final output tile, so the
                # idle runs strictly AFTER the gemm instead of alongside it,
                # and the per-iteration span is gemm_span + idle_span.
                # Keeps average chip power low so duty-cycled benchmarks see
                # the unthrottled PE clock.
                if idle:
                    idle_t = wpool.tile([1, 8], mybir.dt.float16, name="idle_t")
                    if last_ot[0] is not None:
                        nc.sync.dma_start(idle_t[:], last_ot[0][0:1, 0:8])
                    for i in range(idle):
                        if i % 2 == 0:
                            nc.sync.dma_start(pong[:], idle_t[:])
                        else:
                            nc.sync.dma_start(idle_t[:], ping[:])
    nc.compile()
    return nc


def _get_nc(repeats=1, loop=0, idle=0):
    key = (repeats, loop, idle, BLOCKS, X_BUFS, FP8, K16T, OUT_B, WARMUP)
    if key not in _cache:
        _cache[key] = _build_nc(repeats, loop, idle)
    return _cache[key]


def make_in_maps(inputs, weight):
    """Per-core input tensors: transposed, scaled, split fp16/fp8."""
    import ml_dtypes

    f8 = ml_dtypes.float8_e4m3
    in_maps = []
    for e in range(E):
        x_e = inputs[e * S:(e + 1) * S, :]        # [S, I] fp32
        w_e = weight[e] * WSCALE                  # [O, I] fp32, pre-scaled
        m = {
            "xT": np.ascontiguousarray(x_e[:, :K16].T.astype(np.float16)),
            "wT": np.ascontiguousarray(w_e[:, :K16].T.astype(np.float16)),
        }
        if FP8:
            m["x8T"] = np.ascontiguousarray(x_e[:, K16:].T.astype(f8))
            m["w8T"] = np.ascontiguousarray(w_e[:, K16:].T.astype(f8))
        in_maps.append(m)
    return in_maps


def run(inputs, weight, trace=False, repeats=1, loop=0):
    """Shard, run on 8 cores, gather. Returns (out, BassKernelResults)."""
    from concourse.bass_utils import run_bass_kernel_spmd

    nc = _get_nc(repeats, loop)
    in_maps = make_in_maps(inputs, weight)
    res = run_bass_kernel_spmd(nc, in_maps, list(range(E)), trace=trace)
    outs = [res.results[e]["out"] for e in range(E)]
    out = np.concatenate([o.astype(np.float32) for o in outs], axis=0)
    return out, res


def kernel(inputs, weight, expert_size):
    inputs = np.asarray(inputs, dtype=np.float32)
    weight = np.asarray(weight, dtype=np.float32)
    assert inputs.shape == (E * S, I) and weight.shape == (E, O, I)
    assert int(expert_size) == S
    out, _ = run(inputs, weight, trace=False)
    return out
